# revision 17
# baseline (speedup 1.0000x reference)
"""Trainium2 Bass kernel for the E80 gated-recurrence cell (nn_CUDAE80Cell).

Reference math per timestep t (per batch, n=64):
    kvqm = einsum('tbd,nd->tbn', x, W)        k,v,q,m = split(kvqm)
    vk = v_t k_t^T ;  A = sigmoid(vk + B_S) ; S = A*S + (1-A)*vk
    Sq = S q_t     ;  sm = Sq m_t^T ; G = sigmoid(sm + B_M)
    M  = G*M + (1-G)*sm ; out_t = M q_t

Distribution: batch 16 -> 2 per core x 8 cores (pure data parallel, no
collectives). Per core the 2 batches are merged into the 128-partition dim.

Per-core structure:
  * Projections on PE with x transposed on-chip (PE transpose, ACT evictions).
  * State tiles S_i[(b,j), t] (i=0..63); the time recurrence is ONE
    `tensor_tensor_scan` (state = A*state - Cn, Cn = (A-1)*vk) per (i,chunk).
  * v/Sq broadcast across partitions: K=128 PE matmul with a per-i 0/1
    selection weight; exact fp32 via bf16 hi+lo accumulated in PSUM.
  * i-iterations processed in groups of G: one DVE multiply / one
    scalar_tensor_tensor per group (stride-0 broadcast AP for the shared
    k/m operand), amortizing per-instruction overhead.
  * Sigmoids on ACT; when B_S/B_M are constant (the shipped setup), one
    batched sigmoid per group with immediate bias, else per-i with a
    per-partition bias column of B^T.
  * Sq_t / out_t matvecs: per-t PE matmuls contracting (b,j) against a
    zero-padded block-diagonal Q (QD).
"""

import sys

import numpy as np

if "/opt/trn_rl_repo" not in sys.path:
    sys.path.insert(0, "/opt/trn_rl_repo")

from concourse import bacc, bass, mybir, tile  # noqa: E402
from concourse.bass_utils import run_bass_kernel_spmd  # noqa: E402

FP32 = mybir.dt.float32
BF16 = mybir.dt.bfloat16
AF = mybir.ActivationFunctionType
ALU = mybir.AluOpType

T_FULL, B_FULL, D, N = 2048, 16, 1024, 64
NCORES = 8
BL = B_FULL // NCORES  # 2 local batches


def _bcast_free(ap, g):
    """Insert a stride-0 free dim of size g right after the partition dim."""
    pat = [list(ap.ap[0]), [0, g]] + [list(x) for x in ap.ap[1:]]
    return bass.AP(ap.tensor, ap.offset, pat)


def build_nc(T=T_FULL, TC=256, G=4, const_bias=None, repeat=1, gp_mult=False):
    assert T % TC == 0 and TC % 128 == 0 and 64 % G == 0
    n_chunks = T // TC
    n_tb = TC // 128

    nc = bacc.Bacc("TRN2", target_bir_lowering=False, debug=False)

    x = nc.dram_tensor("x", (T, BL, D), FP32, kind="ExternalInput")
    S0 = nc.dram_tensor("S0", (BL, N, N), FP32, kind="ExternalInput")
    M0 = nc.dram_tensor("M0", (BL, N, N), FP32, kind="ExternalInput")
    W = nc.dram_tensor("W", (4 * N, D), FP32, kind="ExternalInput")
    BS = nc.dram_tensor("BS", (N, N), FP32, kind="ExternalInput")
    BM = nc.dram_tensor("BM", (N, N), FP32, kind="ExternalInput")
    outs = nc.dram_tensor("outs", (T, BL, N), FP32, kind="ExternalOutput")
    Sf = nc.dram_tensor("Sf", (BL, N, N), FP32, kind="ExternalOutput")
    Mf = nc.dram_tensor("Mf", (BL, N, N), FP32, kind="ExternalOutput")

    ident = nc.inline_tensor(np.eye(128, dtype=np.float32), "ident128")
    # sel[p, i, (b,j)] = 1 iff p == b*64 + i : K=128 broadcast weights
    sel_np = np.zeros((128, 64, 128), dtype=np.float32)
    for b2 in range(2):
        for i2 in range(64):
            sel_np[b2 * 64 + i2, i2, b2 * 64 : b2 * 64 + 64] = 1.0
    sel = nc.inline_tensor(sel_np.astype(np.dtype("bfloat16")), "sel")

    from contextlib import ExitStack

    with tile.TileContext(nc) as tc, ExitStack() as es:
        cpool = es.enter_context(tc.tile_pool(name="const", bufs=1))
        xnpool = es.enter_context(tc.tile_pool(name="xnat", bufs=2))
        xtpool = es.enter_context(tc.tile_pool(name="xt", bufs=3))
        ppool = es.enter_context(tc.tile_pool(name="proj", bufs=1))
        gpool = es.enter_context(tc.tile_pool(name="gate", bufs=2))
        spool = es.enter_context(tc.tile_pool(name="state", bufs=1))
        stgpool = es.enter_context(tc.tile_pool(name="stg", bufs=2))
        pp_tp = es.enter_context(tc.tile_pool(name="ps_tp", bufs=1, space="PSUM"))
        pp_proj = es.enter_context(tc.tile_pool(name="ps_proj", bufs=1, space="PSUM"))
        pp_bc = es.enter_context(tc.tile_pool(name="ps_bc", bufs=2, space="PSUM"))
        pp_mv = es.enter_context(tc.tile_pool(name="ps_mv", bufs=1, space="PSUM"))

        id_sb = cpool.tile([128, 128], FP32, tag="ident")
        nc.sync.dma_start(out=id_sb[:, :], in_=ident[:, :])
        sel_sb = cpool.tile([128, 64, 128], BF16, tag="sel")
        nc.sync.dma_start(out=sel_sb[:, :, :], in_=sel[:, :, :])

        # ---- W^T in two column orders (heads land at final partitions) -----
        # per d-chunk columns: wt_a = [k|v|m|q] (batch 0), wt_b = [v|k|q|m].
        wt_a = cpool.tile([128, 8, 256], FP32, tag="wt_a")
        wt_b = cpool.tile([128, 8, 256], FP32, tag="wt_b")
        order_a = [0, 1, 3, 2]
        order_b = [1, 0, 2, 3]
        for ft in range(2):
            wn = xnpool.tile([128, 1024], FP32, tag="xnat0")
            nc.sync.dma_start(out=wn[:, :], in_=W[ft * 128 : (ft + 1) * 128, :])
            for dc in range(8):
                pt = pp_tp.tile([128, 512], FP32, tag="tp", name=f"ptw_{ft}_{dc}")
                nc.tensor.transpose(
                    pt[:, 0:128], wn[:, dc * 128 : (dc + 1) * 128], id_sb[:, :]
                )
                for half in range(2):
                    head = ft * 2 + half
                    pa = order_a.index(head)
                    pb = order_b.index(head)
                    nc.scalar.copy(
                        wt_a[:, dc, pa * 64 : pa * 64 + 64],
                        pt[:, half * 64 : half * 64 + 64],
                    )
                    nc.vector.tensor_copy(
                        wt_b[:, dc, pb * 64 : pb * 64 + 64],
                        pt[:, half * 64 : half * 64 + 64],
                    )

        # ---- B_S^T / B_M^T replicated over b: bst[(b,j), i] = B_S[i, j] ----
        bst = cpool.tile([128, 64], FP32, tag="bst")
        bmt = cpool.tile([128, 64], FP32, tag="bmt")
        if const_bias is not None:
            nc.vector.memset(bst[:, 0:1], float(const_bias[0]))
            nc.vector.memset(bmt[:, 0:1], float(const_bias[1]))
        if const_bias is None:
            for src, dst in ((BS, bst), (BM, bmt)):
                bn = stgpool.tile([64, 64], FP32, tag="stg_small")
                nc.sync.dma_start(out=bn[:, :], in_=src[:, :])
                pt = pp_tp.tile([128, 512], FP32, tag="tp", name=f"ptb_{dst.name}")
                nc.tensor.transpose(pt[0:64, 0:64], bn[:, :], id_sb[0:64, 0:64])
                nc.scalar.copy(dst[0:64, :], pt[0:64, 0:64])
                st = stgpool.tile([64, 64], FP32, tag="stg_small2")
                nc.vector.tensor_copy(st[:, :], pt[0:64, 0:64])
                nc.sync.dma_start(out=dst[64:128, :], in_=st[:, :])

        # ---- S0/M0 -> slast/mlast [(b,j), i] -------------------------------
        slast = cpool.tile([128, 64], FP32, tag="slast")
        mlast = cpool.tile([128, 64], FP32, tag="mlast")
        for src, dst in ((S0, slast), (M0, mlast)):
            s0n = stgpool.tile([128, 64], FP32, tag="stg_s0")
            nc.sync.dma_start(out=s0n[:, :], in_=src[:, :, :])
            for b in range(2):
                pt = pp_tp.tile([128, 512], FP32, tag="tp", name=f"pt0_{dst.name}{b}")
                nc.tensor.transpose(
                    pt[0:64, 0:64],
                    s0n[b * 64 : b * 64 + 64, :],
                    id_sb[b * 64 : b * 64 + 64, b * 64 : b * 64 + 64],
                )
                if b == 0:
                    nc.scalar.copy(dst[0:64, :], pt[0:64, 0:64])
                else:
                    st = stgpool.tile([64, 64], FP32, tag="stg_small2")
                    nc.vector.tensor_copy(st[:, :], pt[0:64, 0:64])
                    nc.sync.dma_start(out=dst[64:128, :], in_=st[:, :])

        for cc in range(n_chunks * repeat):
            c = cc
            t0 = (cc % n_chunks) * TC

            # ---------- projections, one batch at a time ----------
            pjs = {}
            for b in range(2):
                wt = wt_a if b == 0 else wt_b
                ps_kv = pp_proj.tile([128, TC], FP32, tag="pjkv", name=f"pskv{b}_{c}")
                ps_qm = pp_proj.tile([128, TC], FP32, tag="pjqm", name=f"psqm{b}_{c}")
                xns = []
                for tb in range(n_tb):
                    xn = xnpool.tile([128, 1024], FP32, tag=f"xnat{tb}")
                    r0 = t0 + tb * 128
                    nc.sync.dma_start(out=xn[:, :], in_=x[r0 : r0 + 128, b, :])
                    xns.append(xn)
                for dc in range(8):
                    xt = xtpool.tile([128, TC], FP32, tag="xt")
                    pt = pp_tp.tile([128, 512], FP32, tag="tp", name=f"ptx{b}_{c}_{dc}")
                    for tb in range(n_tb):
                        nc.tensor.transpose(
                            pt[:, tb * 128 : tb * 128 + 128],
                            xns[tb][:, dc * 128 : (dc + 1) * 128],
                            id_sb[:, :],
                        )
                    nc.scalar.copy(xt[:, :], pt[:, 0:TC])
                    nc.tensor.matmul(
                        ps_kv[:, :], wt[:, dc, 0:128], xt[:, :],
                        start=(dc == 0), stop=(dc == 7),
                    )
                    nc.tensor.matmul(
                        ps_qm[:, :], wt[:, dc, 128:256], xt[:, :],
                        start=(dc == 0), stop=(dc == 7),
                    )
                pjs[b] = (ps_kv, ps_qm)

            # ---------- evictions into head layouts ----------
            k_t = ppool.tile([128, TC], FP32, tag="k_t")
            m_t = ppool.tile([128, TC], FP32, tag="m_t")
            nc.scalar.copy(k_t[0:64, :], pjs[0][0][0:64, :])
            nc.scalar.copy(k_t[64:128, :], pjs[1][0][64:128, :])
            nc.scalar.copy(m_t[0:64, :], pjs[0][1][0:64, :])
            nc.scalar.copy(m_t[64:128, :], pjs[1][1][64:128, :])

            q_t = ppool.tile([128, TC], FP32, tag="q_t")
            stq0 = stgpool.tile([64, TC], FP32, tag="stq0")
            stq1 = stgpool.tile([64, TC], FP32, tag="stq1")
            nc.scalar.copy(stq0[:, :], pjs[0][1][64:128, :])
            nc.scalar.copy(stq1[:, :], pjs[1][1][0:64, :])
            nc.sync.dma_start(out=q_t[0:64, :], in_=stq0[:, :])
            nc.sync.dma_start(out=q_t[64:128, :], in_=stq1[:, :])

            v_t = ppool.tile([128, TC], FP32, tag="v_t")
            stv0 = stgpool.tile([64, TC], FP32, tag="stv0")
            stv1 = stgpool.tile([64, TC], FP32, tag="stv1")
            nc.scalar.copy(stv0[:, :], pjs[0][0][64:128, :])
            nc.scalar.copy(stv1[:, :], pjs[1][0][0:64, :])
            nc.sync.dma_start(out=v_t[0:64, :], in_=stv0[:, :])
            nc.sync.dma_start(out=v_t[64:128, :], in_=stv1[:, :])

            # hi/lo split of v for exact bf16 broadcast
            v_hi = ppool.tile([128, TC], BF16, tag="v_hi")
            v_lo = ppool.tile([128, TC], BF16, tag="v_lo")
            nc.scalar.copy(v_hi[:, :], v_t[:, :])
            nc.vector.tensor_sub(v_lo[:, :], v_t[:, :], v_hi[:, :])

            # QD[(b,j), t, b'] = q[t, b, j] * (b == b')
            qd = ppool.tile([128, TC, 2], FP32, tag="qd")
            nc.vector.memset(qd[:, :, :], 0.0)
            nc.vector.tensor_copy(qd[0:64, :, 0], q_t[0:64, :])
            nc.vector.tensor_copy(qd[64:128, :, 1], q_t[64:128, :])

            # ---------- S phase ----------
            s_all = spool.tile([128, 64, TC], FP32, tag="state", name=f"s_all_{c}")
            for ig in range(0, 64, G):
                vbc = pp_bc.tile([128, G, TC], FP32, tag="vbc", name=f"vbcS{c}_{ig}")
                for g in range(G):
                    nc.tensor.matmul(
                        vbc[:, g, :], sel_sb[:, ig + g, :], v_hi[:, :],
                        start=True, stop=False,
                    )
                    nc.tensor.matmul(
                        vbc[:, g, :], sel_sb[:, ig + g, :], v_lo[:, :],
                        start=False, stop=True,
                    )
                vk = gpool.tile([128, G, TC], FP32, tag="vk")
                if gp_mult:
                    vbs = gpool.tile([128, G, TC], FP32, tag="vbs")
                    nc.scalar.copy(vbs[:, :, :], vbc[:, :, :])
                    nc.gpsimd.tensor_tensor(
                        vk[:, :, :], _bcast_free(k_t[:, :], G), vbs[:, :, :],
                        op=ALU.mult,
                    )
                else:
                    nc.vector.tensor_tensor(
                        vk[:, :, :], _bcast_free(k_t[:, :], G), vbc[:, :, :],
                        op=ALU.mult,
                    )
                a_g = gpool.tile([128, G, TC], FP32, tag="a_g")
                if const_bias is not None:
                    nc.scalar.activation(
                        a_g[:, :, :], vk[:, :, :], AF.Sigmoid,
                        bias=bst[:, 0:1], scale=1.0,
                    )
                else:
                    for g in range(G):
                        nc.scalar.activation(
                            a_g[:, g, :], vk[:, g, :], AF.Sigmoid,
                            bias=bst[:, ig + g : ig + g + 1], scale=1.0,
                        )
                cn = gpool.tile([128, G, TC], FP32, tag="cn")
                nc.vector.scalar_tensor_tensor(
                    cn[:, :, :], a_g[:, :, :], 1.0, vk[:, :, :],
                    op0=ALU.subtract, op1=ALU.mult,
                )
                for g in range(G):
                    i = ig + g
                    nc.vector.tensor_tensor_scan(
                        s_all[:, i, :], a_g[:, g, :], cn[:, g, :],
                        slast[:, i : i + 1], op0=ALU.mult, op1=ALU.subtract,
                    )
            nc.vector.tensor_copy(slast[:, :], s_all[:, :, TC - 1])

            # ---------- Sq matvecs ----------
            ps_sq = pp_mv.tile([64, TC, 2], FP32, tag="mv", name=f"ps_sq_{c}")
            for t in range(TC):
                nc.tensor.matmul(
                    ps_sq[:, t, :], s_all[:, :, t], qd[:, t, :], start=True, stop=True
                )
            sq2 = ppool.tile([128, TC], FP32, tag="sq2")
            nc.scalar.copy(sq2[0:64, :], ps_sq[:, :, 0])
            stq = stgpool.tile([64, TC], FP32, tag="st_sq1")
            nc.scalar.copy(stq[:, :], ps_sq[:, :, 1])
            nc.sync.dma_start(out=sq2[64:128, :], in_=stq[:, :])
            sq_hi = ppool.tile([128, TC], BF16, tag="sq_hi")
            sq_lo = ppool.tile([128, TC], BF16, tag="sq_lo")
            nc.scalar.copy(sq_hi[:, :], sq2[:, :])
            nc.vector.tensor_sub(sq_lo[:, :], sq2[:, :], sq_hi[:, :])

            # ---------- M phase ----------
            m_all = spool.tile([128, 64, TC], FP32, tag="state", name=f"m_all_{c}")
            for ig in range(0, 64, G):
                sqbc = pp_bc.tile([128, G, TC], FP32, tag="vbc", name=f"vbcM{c}_{ig}")
                for g in range(G):
                    nc.tensor.matmul(
                        sqbc[:, g, :], sel_sb[:, ig + g, :], sq_hi[:, :],
                        start=True, stop=False,
                    )
                    nc.tensor.matmul(
                        sqbc[:, g, :], sel_sb[:, ig + g, :], sq_lo[:, :],
                        start=False, stop=True,
                    )
                sm = gpool.tile([128, G, TC], FP32, tag="vk")
                if gp_mult:
                    sbs = gpool.tile([128, G, TC], FP32, tag="vbs")
                    nc.scalar.copy(sbs[:, :, :], sqbc[:, :, :])
                    nc.gpsimd.tensor_tensor(
                        sm[:, :, :], _bcast_free(m_t[:, :], G), sbs[:, :, :],
                        op=ALU.mult,
                    )
                else:
                    nc.vector.tensor_tensor(
                        sm[:, :, :], _bcast_free(m_t[:, :], G), sqbc[:, :, :],
                        op=ALU.mult,
                    )
                g_g = gpool.tile([128, G, TC], FP32, tag="a_g")
                if const_bias is not None:
                    nc.scalar.activation(
                        g_g[:, :, :], sm[:, :, :], AF.Sigmoid,
                        bias=bmt[:, 0:1], scale=1.0,
                    )
                else:
                    for g in range(G):
                        nc.scalar.activation(
                            g_g[:, g, :], sm[:, g, :], AF.Sigmoid,
                            bias=bmt[:, ig + g : ig + g + 1], scale=1.0,
                        )
                cnm = gpool.tile([128, G, TC], FP32, tag="cn")
                nc.vector.scalar_tensor_tensor(
                    cnm[:, :, :], g_g[:, :, :], 1.0, sm[:, :, :],
                    op0=ALU.subtract, op1=ALU.mult,
                )
                for g in range(G):
                    i = ig + g
                    nc.vector.tensor_tensor_scan(
                        m_all[:, i, :], g_g[:, g, :], cnm[:, g, :],
                        mlast[:, i : i + 1], op0=ALU.mult, op1=ALU.subtract,
                    )
            nc.vector.tensor_copy(mlast[:, :], m_all[:, :, TC - 1])

            # ---------- out matvecs ----------
            ps_out = pp_mv.tile([64, TC, 2], FP32, tag="mv", name=f"ps_out_{c}")
            for t in range(TC):
                nc.tensor.matmul(
                    ps_out[:, t, :], m_all[:, :, t], qd[:, t, :], start=True, stop=True
                )
            st_out = stgpool.tile([64, TC * 2], FP32, tag="st_out")
            nc.scalar.copy(st_out[:, :], ps_out[:, :, :])
            for blk in range(TC * 2 // 128):
                pt2 = pp_bc.tile([128, G, TC], FP32, tag="vbc", name=f"pt2_{c}_{blk}")
                nc.tensor.transpose(
                    pt2[:, 0, 0:64],
                    st_out[:, blk * 128 : (blk + 1) * 128],
                    id_sb[0:64, 0:64],
                )
                so2 = stgpool.tile([128, 64], FP32, tag="st_out2")
                nc.scalar.copy(so2[:, :], pt2[:, 0, 0:64])
                trow = t0 + blk * 64
                nc.sync.dma_start(out=outs[trow : trow + 64, :, :], in_=so2[:, :])

        # ---------- final states ----------
        for src, dst in ((slast, Sf), (mlast, Mf)):
            stf = stgpool.tile([64, 2, 64], FP32, tag="st_fin")
            for b in range(2):
                ptf = pp_bc.tile(
                    [128, G, TC], FP32, tag="vbc", name=f"ptf_{dst.name}{b}"
                )
                nc.tensor.transpose(
                    ptf[0:64, 0, 0:64],
                    src[b * 64 : b * 64 + 64, :],
                    id_sb[b * 64 : b * 64 + 64, b * 64 : b * 64 + 64],
                )
                nc.scalar.copy(stf[:, b, :], ptf[0:64, 0, 0:64])
            nc.sync.dma_start(
                out=dst[:, :, :].transpose([1, 0, 2]), in_=stf[:, :, :]
            )

    nc.compile()
    return nc


_BUILT = {}


def _get_built(T=T_FULL, TC=256, const_bias=None):
    key = (T, TC, const_bias)
    if key not in _BUILT:
        _BUILT[key] = build_nc(T, TC, const_bias=const_bias, gp_mult=True)
    return _BUILT[key]


def make_in_maps(x, S0, M0, W_kvqm, B_S, B_M):
    in_maps = []
    for c in range(NCORES):
        bs = slice(c * BL, (c + 1) * BL)
        in_maps.append(
            {
                "x": np.ascontiguousarray(x[:, bs, :], dtype=np.float32),
                "S0": np.ascontiguousarray(S0[bs], dtype=np.float32),
                "M0": np.ascontiguousarray(M0[bs], dtype=np.float32),
                "W": np.ascontiguousarray(W_kvqm, dtype=np.float32),
                "BS": np.ascontiguousarray(B_S, dtype=np.float32),
                "BM": np.ascontiguousarray(B_M, dtype=np.float32),
            }
        )
    return in_maps


def assemble(results, T):
    outs = np.empty((T, B_FULL, N), dtype=np.float32)
    Sf = np.empty((B_FULL, N, N), dtype=np.float32)
    Mf = np.empty((B_FULL, N, N), dtype=np.float32)
    for c in range(NCORES):
        bs = slice(c * BL, (c + 1) * BL)
        outs[:, bs, :] = results[c]["outs"]
        Sf[bs] = results[c]["Sf"]
        Mf[bs] = results[c]["Mf"]
    return outs, Sf, Mf


def _const_or_none(arr):
    arr = np.asarray(arr)
    v = arr.flat[0]
    return float(v) if np.all(arr == v) else None


def kernel(x, S0, M0, W_kvqm, B_S, B_M):
    x = np.asarray(x, dtype=np.float32)
    T = x.shape[0]
    bs_c = _const_or_none(B_S)
    bm_c = _const_or_none(B_M)
    const_bias = (bs_c, bm_c) if (bs_c is not None and bm_c is not None) else None
    nc = _get_built(T=T, const_bias=const_bias)
    in_maps = make_in_maps(x, S0, M0, W_kvqm, B_S, B_M)
    res = run_bass_kernel_spmd(nc, in_maps, core_ids=list(range(NCORES)))
    return assemble(res.results, T)


# revision 22
# speedup vs baseline: 1.0192x; 1.0192x over previous
"""Trainium2 Bass kernel for the E80 gated-recurrence cell (nn_CUDAE80Cell).

Reference math per timestep t (per batch, n=64):
    kvqm = einsum('tbd,nd->tbn', x, W)        k,v,q,m = split(kvqm)
    vk = v_t k_t^T ;  A = sigmoid(vk + B_S) ; S = A*S + (1-A)*vk
    Sq = S q_t     ;  sm = Sq m_t^T ; G = sigmoid(sm + B_M)
    M  = G*M + (1-G)*sm ; out_t = M q_t

Distribution: batch 16 -> 2 per core x 8 cores (pure data parallel, no
collectives). Per core the 2 batches are merged into the 128-partition dim.

Per-core structure:
  * Projections on PE with x transposed on-chip (PE transpose, ACT evictions).
  * State tiles S_i[(b,j), t] (i=0..63); the time recurrence is ONE
    `tensor_tensor_scan` (state = A*state - Cn, Cn = (A-1)*vk) per (i,chunk).
  * v/Sq broadcast across partitions: K=128 PE matmul with a per-i 0/1
    selection weight; exact fp32 via bf16 hi+lo accumulated in PSUM.
  * i-iterations processed in groups of G: one DVE multiply / one
    scalar_tensor_tensor per group (stride-0 broadcast AP for the shared
    k/m operand), amortizing per-instruction overhead.
  * Sigmoids on ACT; when B_S/B_M are constant (the shipped setup), one
    batched sigmoid per group with immediate bias, else per-i with a
    per-partition bias column of B^T.
  * Sq_t / out_t matvecs: per-t PE matmuls contracting (b,j) against a
    zero-padded block-diagonal Q (QD).
"""

import sys

import numpy as np

if "/opt/trn_rl_repo" not in sys.path:
    sys.path.insert(0, "/opt/trn_rl_repo")

from concourse import bacc, bass, mybir, tile  # noqa: E402
from concourse.bass_utils import run_bass_kernel_spmd  # noqa: E402

FP32 = mybir.dt.float32
BF16 = mybir.dt.bfloat16
AF = mybir.ActivationFunctionType
ALU = mybir.AluOpType

T_FULL, B_FULL, D, N = 2048, 16, 1024, 64
NCORES = 8
BL = B_FULL // NCORES  # 2 local batches


def _bcast_free(ap, g):
    """Insert a stride-0 free dim of size g right after the partition dim."""
    pat = [list(ap.ap[0]), [0, g]] + [list(x) for x in ap.ap[1:]]
    return bass.AP(ap.tensor, ap.offset, pat)


def _flat2d(ap):
    """Collapse all free dims of a contiguous AP into one (for the scan)."""
    n = 1
    for step, cnt in ap.ap[1:]:
        n *= cnt
    return bass.AP(ap.tensor, ap.offset, [list(ap.ap[0]), [1, n]])


def build_nc(T=T_FULL, TC=256, G=4, const_bias=None, repeat=1, gp_mult=False, mv_split=False, state_bufs=1, bc_bufs=2, proj_bufs=1):
    assert T % TC == 0 and TC % 128 == 0 and 64 % G == 0
    n_chunks = T // TC
    n_tb = TC // 128

    nc = bacc.Bacc("TRN2", target_bir_lowering=False, debug=False)

    x = nc.dram_tensor("x", (T, BL, D), FP32, kind="ExternalInput")
    S0 = nc.dram_tensor("S0", (BL, N, N), FP32, kind="ExternalInput")
    M0 = nc.dram_tensor("M0", (BL, N, N), FP32, kind="ExternalInput")
    W = nc.dram_tensor("W", (4 * N, D), FP32, kind="ExternalInput")
    BS = nc.dram_tensor("BS", (N, N), FP32, kind="ExternalInput")
    BM = nc.dram_tensor("BM", (N, N), FP32, kind="ExternalInput")
    outs = nc.dram_tensor("outs", (T, BL, N), FP32, kind="ExternalOutput")
    Sf = nc.dram_tensor("Sf", (BL, N, N), FP32, kind="ExternalOutput")
    Mf = nc.dram_tensor("Mf", (BL, N, N), FP32, kind="ExternalOutput")

    ident = nc.inline_tensor(np.eye(128, dtype=np.float32), "ident128")
    # sel[p, i, (b,j)] = 1 iff p == b*64 + i : K=128 broadcast weights
    sel_np = np.zeros((128, 64, 128), dtype=np.float32)
    for b2 in range(2):
        for i2 in range(64):
            sel_np[b2 * 64 + i2, i2, b2 * 64 : b2 * 64 + 64] = 1.0
    sel = nc.inline_tensor(sel_np.astype(np.dtype("bfloat16")), "sel")

    from contextlib import ExitStack

    with tile.TileContext(nc) as tc, ExitStack() as es:
        cpool = es.enter_context(tc.tile_pool(name="const", bufs=1))
        xnpool = es.enter_context(tc.tile_pool(name="xnat", bufs=2))
        xtpool = es.enter_context(tc.tile_pool(name="xt", bufs=3))
        ppool = es.enter_context(tc.tile_pool(name="proj", bufs=proj_bufs))
        gpool = es.enter_context(tc.tile_pool(name="gate", bufs=2))
        spool = es.enter_context(tc.tile_pool(name="state", bufs=state_bufs))
        stgpool = es.enter_context(tc.tile_pool(name="stg", bufs=2))
        pp_tp = es.enter_context(tc.tile_pool(name="ps_tp", bufs=1, space="PSUM"))
        pp_proj = es.enter_context(tc.tile_pool(name="ps_proj", bufs=1, space="PSUM"))
        pp_bc = es.enter_context(tc.tile_pool(name="ps_bc", bufs=bc_bufs, space="PSUM"))
        pp_mv = es.enter_context(tc.tile_pool(name="ps_mv", bufs=1, space="PSUM"))

        id_sb = cpool.tile([128, 128], FP32, tag="ident")
        nc.sync.dma_start(out=id_sb[:, :], in_=ident[:, :])
        sel_sb = cpool.tile([128, 64, 128], BF16, tag="sel")
        nc.sync.dma_start(out=sel_sb[:, :, :], in_=sel[:, :, :])

        # ---- W^T in two column orders (heads land at final partitions) -----
        # per d-chunk columns: wt_a = [k|v|m|q] (batch 0), wt_b = [v|k|q|m].
        wt_a = cpool.tile([128, 8, 256], FP32, tag="wt_a")
        wt_b = cpool.tile([128, 8, 256], FP32, tag="wt_b")
        order_a = [0, 1, 3, 2]
        order_b = [1, 0, 2, 3]
        for ft in range(2):
            wn = xnpool.tile([128, 1024], FP32, tag="xnat0")
            nc.sync.dma_start(out=wn[:, :], in_=W[ft * 128 : (ft + 1) * 128, :])
            for dc in range(8):
                pt = pp_tp.tile([128, 512], FP32, tag="tp", name=f"ptw_{ft}_{dc}")
                nc.tensor.transpose(
                    pt[:, 0:128], wn[:, dc * 128 : (dc + 1) * 128], id_sb[:, :]
                )
                for half in range(2):
                    head = ft * 2 + half
                    pa = order_a.index(head)
                    pb = order_b.index(head)
                    nc.scalar.copy(
                        wt_a[:, dc, pa * 64 : pa * 64 + 64],
                        pt[:, half * 64 : half * 64 + 64],
                    )
                    nc.vector.tensor_copy(
                        wt_b[:, dc, pb * 64 : pb * 64 + 64],
                        pt[:, half * 64 : half * 64 + 64],
                    )

        # ---- B_S^T / B_M^T replicated over b: bst[(b,j), i] = B_S[i, j] ----
        bst = cpool.tile([128, 64], FP32, tag="bst")
        bmt = cpool.tile([128, 64], FP32, tag="bmt")
        if const_bias is not None:
            nc.vector.memset(bst[:, 0:1], float(const_bias[0]))
            nc.vector.memset(bmt[:, 0:1], float(const_bias[1]))
        if const_bias is None:
            for src, dst in ((BS, bst), (BM, bmt)):
                bn = stgpool.tile([64, 64], FP32, tag="stg_small")
                nc.sync.dma_start(out=bn[:, :], in_=src[:, :])
                pt = pp_tp.tile([128, 512], FP32, tag="tp", name=f"ptb_{dst.name}")
                nc.tensor.transpose(pt[0:64, 0:64], bn[:, :], id_sb[0:64, 0:64])
                nc.scalar.copy(dst[0:64, :], pt[0:64, 0:64])
                st = stgpool.tile([64, 64], FP32, tag="stg_small2")
                nc.vector.tensor_copy(st[:, :], pt[0:64, 0:64])
                nc.sync.dma_start(out=dst[64:128, :], in_=st[:, :])

        # ---- S0/M0 -> slast/mlast [(b,j), i] -------------------------------
        slast = cpool.tile([128, 64], FP32, tag="slast")
        mlast = cpool.tile([128, 64], FP32, tag="mlast")
        for src, dst in ((S0, slast), (M0, mlast)):
            s0n = stgpool.tile([128, 64], FP32, tag="stg_s0")
            nc.sync.dma_start(out=s0n[:, :], in_=src[:, :, :])
            for b in range(2):
                pt = pp_tp.tile([128, 512], FP32, tag="tp", name=f"pt0_{dst.name}{b}")
                nc.tensor.transpose(
                    pt[0:64, 0:64],
                    s0n[b * 64 : b * 64 + 64, :],
                    id_sb[b * 64 : b * 64 + 64, b * 64 : b * 64 + 64],
                )
                if b == 0:
                    nc.scalar.copy(dst[0:64, :], pt[0:64, 0:64])
                else:
                    st = stgpool.tile([64, 64], FP32, tag="stg_small2")
                    nc.vector.tensor_copy(st[:, :], pt[0:64, 0:64])
                    nc.sync.dma_start(out=dst[64:128, :], in_=st[:, :])

        for cc in range(n_chunks * repeat):
            c = cc
            t0 = (cc % n_chunks) * TC

            # ---------- projections, one batch at a time ----------
            pjs = {}
            for b in range(2):
                wt = wt_a if b == 0 else wt_b
                ps_kv = pp_proj.tile([128, TC], FP32, tag="pjkv", name=f"pskv{b}_{c}")
                ps_qm = pp_proj.tile([128, TC], FP32, tag="pjqm", name=f"psqm{b}_{c}")
                xns = []
                for tb in range(n_tb):
                    xn = xnpool.tile([128, 1024], FP32, tag=f"xnat{tb}")
                    r0 = t0 + tb * 128
                    nc.sync.dma_start(out=xn[:, :], in_=x[r0 : r0 + 128, b, :])
                    xns.append(xn)
                for dc in range(8):
                    xt = xtpool.tile([128, TC], FP32, tag="xt")
                    pt = pp_tp.tile([128, 512], FP32, tag="tp", name=f"ptx{b}_{c}_{dc}")
                    for tb in range(n_tb):
                        nc.tensor.transpose(
                            pt[:, tb * 128 : tb * 128 + 128],
                            xns[tb][:, dc * 128 : (dc + 1) * 128],
                            id_sb[:, :],
                        )
                    nc.scalar.copy(xt[:, :], pt[:, 0:TC])
                    nc.tensor.matmul(
                        ps_kv[:, :], wt[:, dc, 0:128], xt[:, :],
                        start=(dc == 0), stop=(dc == 7),
                    )
                    nc.tensor.matmul(
                        ps_qm[:, :], wt[:, dc, 128:256], xt[:, :],
                        start=(dc == 0), stop=(dc == 7),
                    )
                pjs[b] = (ps_kv, ps_qm)

            # ---------- evictions into head layouts ----------
            k_t = ppool.tile([128, TC], FP32, tag="k_t")
            m_t = ppool.tile([128, TC], FP32, tag="m_t")
            nc.scalar.copy(k_t[0:64, :], pjs[0][0][0:64, :])
            nc.scalar.copy(k_t[64:128, :], pjs[1][0][64:128, :])
            nc.scalar.copy(m_t[0:64, :], pjs[0][1][0:64, :])
            nc.scalar.copy(m_t[64:128, :], pjs[1][1][64:128, :])

            q_t = ppool.tile([128, TC], FP32, tag="q_t")
            stq0 = stgpool.tile([64, TC], FP32, tag="stq0")
            stq1 = stgpool.tile([64, TC], FP32, tag="stq1")
            nc.scalar.copy(stq0[:, :], pjs[0][1][64:128, :])
            nc.scalar.copy(stq1[:, :], pjs[1][1][0:64, :])
            nc.sync.dma_start(out=q_t[0:64, :], in_=stq0[:, :])
            nc.sync.dma_start(out=q_t[64:128, :], in_=stq1[:, :])

            v_t = ppool.tile([128, TC], FP32, tag="v_t")
            stv0 = stgpool.tile([64, TC], FP32, tag="stv0")
            stv1 = stgpool.tile([64, TC], FP32, tag="stv1")
            nc.scalar.copy(stv0[:, :], pjs[0][0][64:128, :])
            nc.scalar.copy(stv1[:, :], pjs[1][0][0:64, :])
            nc.sync.dma_start(out=v_t[0:64, :], in_=stv0[:, :])
            nc.sync.dma_start(out=v_t[64:128, :], in_=stv1[:, :])

            # hi/lo split of v for exact bf16 broadcast
            v_hi = ppool.tile([128, TC], BF16, tag="v_hi")
            v_lo = ppool.tile([128, TC], BF16, tag="v_lo")
            nc.scalar.copy(v_hi[:, :], v_t[:, :])
            nc.vector.tensor_sub(v_lo[:, :], v_t[:, :], v_hi[:, :])

            # QD[(b,j), t, b'] = q[t, b, j] * (b == b')
            qd = ppool.tile([128, TC, 2], FP32, tag="qd")
            nc.vector.memset(qd[:, :, :], 0.0)
            nc.vector.tensor_copy(qd[0:64, :, 0], q_t[0:64, :])
            nc.vector.tensor_copy(qd[64:128, :, 1], q_t[64:128, :])

            # ---------- S phase ----------
            s_all = spool.tile([128, 64, TC + 1], FP32, tag="state", name=f"s_all_{c}")
            for ig in range(0, 64, G):
                vbc = pp_bc.tile([128, G, TC], FP32, tag="vbc", name=f"vbcS{c}_{ig}")
                for g in range(G):
                    nc.tensor.matmul(
                        vbc[:, g, :], sel_sb[:, ig + g, :], v_hi[:, :],
                        start=True, stop=False,
                    )
                    nc.tensor.matmul(
                        vbc[:, g, :], sel_sb[:, ig + g, :], v_lo[:, :],
                        start=False, stop=True,
                    )
                vk = gpool.tile([128, G, TC + 1], FP32, tag="vk")
                if gp_mult:
                    vbs = gpool.tile([128, G, TC], FP32, tag="vbs")
                    nc.scalar.copy(vbs[:, :, :], vbc[:, :, :])
                    nc.gpsimd.tensor_tensor(
                        vk[:, :, 1:], _bcast_free(k_t[:, :], G), vbs[:, :, :],
                        op=ALU.mult,
                    )
                else:
                    nc.vector.tensor_tensor(
                        vk[:, :, 1:], _bcast_free(k_t[:, :], G), vbc[:, :, :],
                        op=ALU.mult,
                    )
                a_g = gpool.tile([128, G, TC + 1], FP32, tag="a_g")
                if const_bias is not None:
                    nc.scalar.activation(
                        a_g[:, :, 1:], vk[:, :, 1:], AF.Sigmoid,
                        bias=bst[:, 0:1], scale=1.0,
                    )
                else:
                    for g in range(G):
                        nc.scalar.activation(
                            a_g[:, g, 1:], vk[:, g, 1:], AF.Sigmoid,
                            bias=bst[:, ig + g : ig + g + 1], scale=1.0,
                        )
                cn = gpool.tile([128, G, TC + 1], FP32, tag="cn")
                nc.vector.scalar_tensor_tensor(
                    cn[:, :, 1:], a_g[:, :, 1:], 1.0, vk[:, :, 1:],
                    op0=ALU.subtract, op1=ALU.mult,
                )
                # reset columns: state <- (0*state) - (-initial) = initial
                nc.vector.memset(a_g[:, :, 0], 0.0)
                nc.vector.tensor_scalar(
                    cn[:, :, 0], slast[:, ig : ig + G], -1.0, None, op0=ALU.mult
                )
                nc.vector.tensor_tensor_scan(
                    _flat2d(s_all[:, ig : ig + G, :]),
                    _flat2d(a_g[:, :, :]),
                    _flat2d(cn[:, :, :]),
                    slast[:, ig : ig + 1],
                    op0=ALU.mult, op1=ALU.subtract,
                )
            nc.vector.tensor_copy(slast[:, :], s_all[:, :, TC])

            # ---------- Sq matvecs ----------
            ps_sq = pp_mv.tile([64, TC, 2], FP32, tag="mv", name=f"ps_sq_{c}")
            if mv_split:
                for t in range(TC):
                    nc.tensor.matmul(
                        ps_sq[0:32, t, :], s_all[:, 0:32, t + 1], qd[:, t, :],
                        start=True, stop=True,
                    )
                    nc.tensor.matmul(
                        ps_sq[32:64, t, :], s_all[:, 32:64, t + 1], qd[:, t, :],
                        start=True, stop=True,
                    )
            else:
                for t in range(TC):
                    nc.tensor.matmul(
                        ps_sq[:, t, :], s_all[:, :, t + 1], qd[:, t, :],
                        start=True, stop=True,
                    )
            sq2 = ppool.tile([128, TC], FP32, tag="sq2")
            nc.scalar.copy(sq2[0:64, :], ps_sq[:, :, 0])
            stq = stgpool.tile([64, TC], FP32, tag="st_sq1")
            nc.scalar.copy(stq[:, :], ps_sq[:, :, 1])
            nc.sync.dma_start(out=sq2[64:128, :], in_=stq[:, :])
            sq_hi = ppool.tile([128, TC], BF16, tag="sq_hi")
            sq_lo = ppool.tile([128, TC], BF16, tag="sq_lo")
            nc.scalar.copy(sq_hi[:, :], sq2[:, :])
            nc.vector.tensor_sub(sq_lo[:, :], sq2[:, :], sq_hi[:, :])

            # ---------- M phase ----------
            m_all = spool.tile([128, 64, TC + 1], FP32, tag="state", name=f"m_all_{c}")
            for ig in range(0, 64, G):
                sqbc = pp_bc.tile([128, G, TC], FP32, tag="vbc", name=f"vbcM{c}_{ig}")
                for g in range(G):
                    nc.tensor.matmul(
                        sqbc[:, g, :], sel_sb[:, ig + g, :], sq_hi[:, :],
                        start=True, stop=False,
                    )
                    nc.tensor.matmul(
                        sqbc[:, g, :], sel_sb[:, ig + g, :], sq_lo[:, :],
                        start=False, stop=True,
                    )
                sm = gpool.tile([128, G, TC + 1], FP32, tag="vk")
                if gp_mult:
                    sbs = gpool.tile([128, G, TC], FP32, tag="vbs")
                    nc.scalar.copy(sbs[:, :, :], sqbc[:, :, :])
                    nc.gpsimd.tensor_tensor(
                        sm[:, :, 1:], _bcast_free(m_t[:, :], G), sbs[:, :, :],
                        op=ALU.mult,
                    )
                else:
                    nc.vector.tensor_tensor(
                        sm[:, :, 1:], _bcast_free(m_t[:, :], G), sqbc[:, :, :],
                        op=ALU.mult,
                    )
                g_g = gpool.tile([128, G, TC + 1], FP32, tag="a_g")
                if const_bias is not None:
                    nc.scalar.activation(
                        g_g[:, :, 1:], sm[:, :, 1:], AF.Sigmoid,
                        bias=bmt[:, 0:1], scale=1.0,
                    )
                else:
                    for g in range(G):
                        nc.scalar.activation(
                            g_g[:, g, 1:], sm[:, g, 1:], AF.Sigmoid,
                            bias=bmt[:, ig + g : ig + g + 1], scale=1.0,
                        )
                cnm = gpool.tile([128, G, TC + 1], FP32, tag="cn")
                nc.vector.scalar_tensor_tensor(
                    cnm[:, :, 1:], g_g[:, :, 1:], 1.0, sm[:, :, 1:],
                    op0=ALU.subtract, op1=ALU.mult,
                )
                nc.vector.memset(g_g[:, :, 0], 0.0)
                nc.vector.tensor_scalar(
                    cnm[:, :, 0], mlast[:, ig : ig + G], -1.0, None, op0=ALU.mult
                )
                nc.vector.tensor_tensor_scan(
                    _flat2d(m_all[:, ig : ig + G, :]),
                    _flat2d(g_g[:, :, :]),
                    _flat2d(cnm[:, :, :]),
                    mlast[:, ig : ig + 1],
                    op0=ALU.mult, op1=ALU.subtract,
                )
            nc.vector.tensor_copy(mlast[:, :], m_all[:, :, TC])

            # ---------- out matvecs ----------
            ps_out = pp_mv.tile([64, TC, 2], FP32, tag="mv", name=f"ps_out_{c}")
            if mv_split:
                for t in range(TC):
                    nc.tensor.matmul(
                        ps_out[0:32, t, :], m_all[:, 0:32, t + 1], qd[:, t, :],
                        start=True, stop=True,
                    )
                    nc.tensor.matmul(
                        ps_out[32:64, t, :], m_all[:, 32:64, t + 1], qd[:, t, :],
                        start=True, stop=True,
                    )
            else:
                for t in range(TC):
                    nc.tensor.matmul(
                        ps_out[:, t, :], m_all[:, :, t + 1], qd[:, t, :],
                        start=True, stop=True,
                    )
            st_out = stgpool.tile([64, TC * 2], FP32, tag="st_out")
            nc.scalar.copy(st_out[:, :], ps_out[:, :, :])
            for blk in range(TC * 2 // 128):
                pt2 = pp_bc.tile([128, G, TC], FP32, tag="vbc", name=f"pt2_{c}_{blk}")
                nc.tensor.transpose(
                    pt2[:, 0, 0:64],
                    st_out[:, blk * 128 : (blk + 1) * 128],
                    id_sb[0:64, 0:64],
                )
                so2 = stgpool.tile([128, 64], FP32, tag="st_out2")
                nc.scalar.copy(so2[:, :], pt2[:, 0, 0:64])
                trow = t0 + blk * 64
                nc.sync.dma_start(out=outs[trow : trow + 64, :, :], in_=so2[:, :])

        # ---------- final states ----------
        for src, dst in ((slast, Sf), (mlast, Mf)):
            stf = stgpool.tile([64, 2, 64], FP32, tag="st_fin")
            for b in range(2):
                ptf = pp_bc.tile(
                    [128, G, TC], FP32, tag="vbc", name=f"ptf_{dst.name}{b}"
                )
                nc.tensor.transpose(
                    ptf[0:64, 0, 0:64],
                    src[b * 64 : b * 64 + 64, :],
                    id_sb[b * 64 : b * 64 + 64, b * 64 : b * 64 + 64],
                )
                nc.scalar.copy(stf[:, b, :], ptf[0:64, 0, 0:64])
            nc.sync.dma_start(
                out=dst[:, :, :].transpose([1, 0, 2]), in_=stf[:, :, :]
            )

    nc.compile()
    return nc


_BUILT = {}


def _get_built(T=T_FULL, TC=256, const_bias=None):
    key = (T, TC, const_bias)
    if key not in _BUILT:
        _BUILT[key] = build_nc(T, TC, const_bias=const_bias, gp_mult=True)
    return _BUILT[key]


def make_in_maps(x, S0, M0, W_kvqm, B_S, B_M):
    in_maps = []
    for c in range(NCORES):
        bs = slice(c * BL, (c + 1) * BL)
        in_maps.append(
            {
                "x": np.ascontiguousarray(x[:, bs, :], dtype=np.float32),
                "S0": np.ascontiguousarray(S0[bs], dtype=np.float32),
                "M0": np.ascontiguousarray(M0[bs], dtype=np.float32),
                "W": np.ascontiguousarray(W_kvqm, dtype=np.float32),
                "BS": np.ascontiguousarray(B_S, dtype=np.float32),
                "BM": np.ascontiguousarray(B_M, dtype=np.float32),
            }
        )
    return in_maps


def assemble(results, T):
    outs = np.empty((T, B_FULL, N), dtype=np.float32)
    Sf = np.empty((B_FULL, N, N), dtype=np.float32)
    Mf = np.empty((B_FULL, N, N), dtype=np.float32)
    for c in range(NCORES):
        bs = slice(c * BL, (c + 1) * BL)
        outs[:, bs, :] = results[c]["outs"]
        Sf[bs] = results[c]["Sf"]
        Mf[bs] = results[c]["Mf"]
    return outs, Sf, Mf


def _const_or_none(arr):
    arr = np.asarray(arr)
    v = arr.flat[0]
    return float(v) if np.all(arr == v) else None


def kernel(x, S0, M0, W_kvqm, B_S, B_M):
    x = np.asarray(x, dtype=np.float32)
    T = x.shape[0]
    bs_c = _const_or_none(B_S)
    bm_c = _const_or_none(B_M)
    const_bias = (bs_c, bm_c) if (bs_c is not None and bm_c is not None) else None
    nc = _get_built(T=T, const_bias=const_bias)
    in_maps = make_in_maps(x, S0, M0, W_kvqm, B_S, B_M)
    res = run_bass_kernel_spmd(nc, in_maps, core_ids=list(range(NCORES)))
    return assemble(res.results, T)
